# revision 1
# baseline (speedup 1.0000x reference)
"""Trainium2 Bass kernel for nn_CapsuleLayer (grouped 5x5 capsule conv + 3-iter
dynamic routing with local softmax), data-parallel over batch N=8 across 8 cores.

Per-core "C layout": spatial positions on SBUF partitions, channels on free dims.
  h = hb_h*16 + p_h   (hb_h in [0,3), p_h in [0,16))
  w = hb_w*8  + p_w   (hb_w in [0,6), p_w in [0,8))
  partition p = p_h*8 + p_w  (128)
  free block hb = hb_h*6 + hb_w  (18)

u_hat: [p=128, (hb=18, ci=8, co=16, do=16)] bf16.  All routing contractions
(ci/co/do) are free-dim ops (tensor_tensor trees, free-broadcast via 0-stride
APs); the 5x5 spatial pools run on small [48, (ci, 52)] transposed tiles via
DMA.  Conv runs on PE as 2 stacked-tap bf16 matmuls (K=128: 4x4 tap block x
di, K=72: remaining 9 taps x di) per (ci, out-half), accumulated in PSUM.
"""

import numpy as np
import ml_dtypes
from contextlib import ExitStack

import concourse.bass as bass
import concourse.tile as tile
from concourse import bacc, mybir
from concourse.bass_utils import run_bass_kernel_spmd

F32 = mybir.dt.float32
BF16 = mybir.dt.bfloat16
AF = mybir.ActivationFunctionType
ALU = mybir.AluOpType
AX = mybir.AxisListType

CI, DI, CO, DO = 8, 8, 16, 16
H = W = 48
HP = WP = 52
HW = H * W
HB = 18
PAD = 2
ROUTING = 3
NEG = -3.0e38

# taps: K1 = (kh,kw) in [0,4)x[0,4): t = kh*4+kw, row t*8+di  -> 128 rows
#       K2 = kh=4,kw=0..5 (5 taps) then kh=0..4,kw=4 (4 taps) -> 72 rows
K1_TAPS = [(kh, kw) for kh in range(4) for kw in range(4)]
K2_TAPS = [(4, kw) for kw in range(5)] + [(kh, 4) for kh in range(4)]


GUARD = 128  # zero guard elements before/after the flat image (h zero-pad)


def _emit(nc):
    u_d = nc.dram_tensor("u", [CI, DI, H, W], BF16, kind="ExternalInput").ap()
    w1_d = nc.dram_tensor("w1", [128, CI, 256], BF16, kind="ExternalInput").ap()
    w2_d = nc.dram_tensor("w2", [72, CI, 256], BF16, kind="ExternalInput").ap()
    r0_d = nc.dram_tensor("r0c", [128, HB], F32, kind="ExternalInput").ap()
    v_d = nc.dram_tensor("v", [128, HB, CO, DO], F32, kind="ExternalOutput").ap()

    with tile.TileContext(nc) as tc, ExitStack() as ctx:
        const = ctx.enter_context(tc.tile_pool(name="const", bufs=1))
        convp = ctx.enter_context(tc.tile_pool(name="convp", bufs=2))
        patp1 = ctx.enter_context(tc.tile_pool(name="patp1", bufs=2))
        patp2 = ctx.enter_context(tc.tile_pool(name="patp2", bufs=2))
        psum = ctx.enter_context(tc.tile_pool(name="psum", bufs=6, space="PSUM"))
        big = ctx.enter_context(tc.tile_pool(name="big", bufs=1))
        ring = ctx.enter_context(tc.tile_pool(name="ring", bufs=1))
        bigf = ctx.enter_context(tc.tile_pool(name="bigf", bufs=1))
        sm = ctx.enter_context(tc.tile_pool(name="sm", bufs=1))
        poolt = ctx.enter_context(tc.tile_pool(name="poolt", bufs=1))
        dpool = ctx.enter_context(tc.tile_pool(name="dpool", bufs=2, space="DRAM"))

        # ---- persistent tiles ----
        uhat = big.tile([128, HB, CI, CO, DO], BF16, name="uhat")
        b_t = big.tile([128, HB, CI, CO], F32, name="b_t")
        w1_t = const.tile([128, CI, 256], BF16, name="w1_t")
        w2_t = const.tile([72, CI, 256], BF16, name="w2_t")
        r0_t = const.tile([128, HB], F32, name="r0_t")
        nc.sync.dma_start(w1_t[:], w1_d[:])
        nc.sync.dma_start(w2_t[:], w2_d[:])
        nc.sync.dma_start(r0_t[:], r0_d[:])

        # pool scratch (pads preset once; interiors overwritten each use)
        mph = poolt.tile([48, WP, CI], F32, name="mph")  # [h, (wpad, ci)]
        mpw = poolt.tile([48, HP, CI], F32, name="mpw")  # [w, (hpad, ci)]
        sph = poolt.tile([48, WP, CI], F32, name="sph")
        spw = poolt.tile([48, HP, CI], F32, name="spw")
        nc.vector.memset(mph[:], NEG)
        nc.vector.memset(mpw[:], NEG)
        nc.vector.memset(sph[:], 0.0)
        nc.vector.memset(spw[:], 0.0)
        eps_t = const.tile([128, 1], F32, name="eps_t")
        nc.vector.memset(eps_t[:], 1e-9)

        # =========== Stage 1: conv -> uhat ===========
        # flat image (pitch 48) with zero guards; tap (kh,kw) = one contiguous
        # 2304-read at offset (kh-2)*48+(kw-2); w-bleed columns zeroed with
        # plain per-tap memsets.  M-block = contiguous flat-hw 128-run, so
        # PSUM partitions are C-partitions (p = hw%128) and evac is contiguous.
        upads = []
        for par in range(2):
            t = convp.tile([DI, GUARD + HW + GUARD], BF16, name=f"upad{par}", bufs=1)
            nc.vector.memset(t[:], 0.0)
            upads.append(t)

        for ci in range(CI):
            uflat = upads[ci % 2]
            nc.sync.dma_start(
                uflat[:, GUARD : GUARD + HW], u_d[ci].rearrange("di h w -> di (h w)")
            )
            pat1 = patp1.tile([128, HW], BF16, name="pat1")
            pat2 = patp2.tile([72, HW], BF16, name="pat2")
            for t, (kh, kw) in enumerate(K1_TAPS):
                off = GUARD + (kh - PAD) * W + (kw - PAD)
                nc.sync.dma_start(
                    pat1[t * 8 : (t + 1) * 8, :], uflat[:, off : off + HW]
                )
            for t, (kh, kw) in enumerate(K2_TAPS):
                off = GUARD + (kh - PAD) * W + (kw - PAD)
                nc.sync.dma_start(
                    pat2[t * 8 : (t + 1) * 8, :], uflat[:, off : off + HW]
                )
            # per-tap w-bleed zeroing via tiny DMAs from the (zero) guard
            for pat, taps in ((pat1, K1_TAPS), (pat2, K2_TAPS)):
                for t, (kh, kw) in enumerate(taps):
                    dw = kw - PAD
                    if dw == 0:
                        continue
                    a = abs(dw)
                    pv = pat[t * 8 : (t + 1) * 8, :].rearrange(
                        "di (h w) -> di h w", h=H
                    )
                    zsrc = uflat[:, 0 : H * a].rearrange(
                        "di (h w) -> di h w", w=a
                    )
                    dst = pv[:, :, W - dw : W] if dw > 0 else pv[:, :, 0:a]
                    nc.sync.dma_start(dst, zsrc)

            for hb in range(HB):
                lhs1 = pat1[:, hb * 128 : (hb + 1) * 128]
                lhs2 = pat2[:, hb * 128 : (hb + 1) * 128]
                ps = psum.tile([128, CO, DO], F32, name="ps")
                nc.tensor.matmul(ps[:], lhs1, w1_t[:, ci, :], start=True, stop=False)
                nc.tensor.matmul(ps[:], lhs2, w2_t[:, ci, :], start=False, stop=True)
                if hb % 2 == 0:
                    nc.scalar.copy(uhat[:, hb, ci], ps[:])
                else:
                    nc.vector.tensor_copy(uhat[:, hb, ci], ps[:])

        # =========== routing helpers ===========
        def mw_int(pwt):
            return pwt[:, PAD : PAD + H, :]

        def pools(src_c, is_max, out_c):
            """src_c [128,(hb,ci)] f32 -> 5x5 'same' window max/sum -> out_c.
            Spatial reorders ride DRAM-side APs (flat hw raster); SBUF-side
            APs stay plain."""
            ph, pw = (mph, mpw) if is_max else (sph, spw)
            op = ALU.max if is_max else ALU.add
            md = dpool.tile([HW, CI], F32, name="pmd", tag="pmd")
            nc.sync.dma_start(
                md[:].rearrange("(hb p) ci -> p hb ci", hb=HB), src_c[:]
            )
            nc.sync.dma_start(
                ph[:, PAD : PAD + W, :],
                md[:].rearrange("(h w) ci -> h w ci", h=H),
            )
            l1 = sm.tile([48, 51, CI], F32, name="pl1", tag="p51")
            nc.vector.tensor_tensor(l1[:], ph[:, 0:51], ph[:, 1:52], op=op)
            l2 = sm.tile([48, 49, CI], F32, name="pl2", tag="p49")
            nc.vector.tensor_tensor(l2[:], l1[:, 0:49], l1[:, 2:51], op=op)
            l3 = sm.tile([48, 48, CI], F32, name="pl3", tag="p48")
            nc.vector.tensor_tensor(l3[:], l2[:, 0:48], ph[:, 4:52], op=op)
            mt = dpool.tile([48, 48, CI], F32, name="pmt", tag="pmt")
            nc.sync.dma_start(mt[:], l3[:])
            nc.sync.dma_start(
                pw[:, PAD : PAD + H, :], mt[:].rearrange("h w ci -> w h ci")
            )
            m1 = sm.tile([48, 51, CI], F32, name="pm1", tag="p51")
            nc.vector.tensor_tensor(m1[:], pw[:, 0:51], pw[:, 1:52], op=op)
            m2 = sm.tile([48, 49, CI], F32, name="pm2", tag="p49")
            nc.vector.tensor_tensor(m2[:], m1[:, 0:49], m1[:, 2:51], op=op)
            m3 = sm.tile([48, 48, CI], F32, name="pm3", tag="p48")
            nc.vector.tensor_tensor(m3[:], m2[:, 0:48], pw[:, 4:52], op=op)
            mo = dpool.tile([HW, CI], F32, name="pmo", tag="pmo")
            nc.sync.dma_start(
                mo[:].rearrange("(h w) ci -> w h ci", h=H), m3[:]
            )
            nc.sync.dma_start(
                out_c[:], mo[:].rearrange("(hb p) ci -> p hb ci", hb=HB)
            )

        def squash(p_c, out_bf, out_f32):
            """p_c [128,(hb,co,do)] f32 -> squash over do."""
            sq = bigf.tile([128, HB, CO, DO], BF16, name="sq", tag="half", bufs=2)
            nc.scalar.activation(sq[:], p_c[:], AF.Square)
            nsq = sm.tile([128, HB, CO], F32, name="nsq")
            nc.vector.tensor_reduce(nsq[:], sq[:], axis=AX.X, op=ALU.add)
            rs = sm.tile([128, HB, CO], F32, name="rs")
            nc.scalar.activation(rs[:], nsq[:], AF.Sqrt, bias=eps_t[:])
            d1 = sm.tile([128, HB, CO], F32, name="d1")
            nc.vector.tensor_scalar_add(d1[:], nsq[:], 1.0)
            d2 = sm.tile([128, HB, CO], F32, name="d2")
            nc.vector.tensor_tensor(d2[:], d1[:], rs[:], op=ALU.mult)
            rd = sm.tile([128, HB, CO], F32, name="rd")
            nc.vector.reciprocal(rd[:], d2[:])
            g2 = sm.tile([128, HB, CO], F32, name="g2")
            nc.vector.tensor_tensor(g2[:], nsq[:], rd[:], op=ALU.mult)
            g_b = g2[:].unsqueeze(3).broadcast_to([128, HB, CO, DO])
            if out_bf is not None:
                nc.vector.tensor_tensor(out_bf[:], p_c[:], g_b, op=ALU.mult)
            if out_f32 is not None:
                nc.vector.tensor_tensor(out_f32[:], p_c[:], g_b, op=ALU.mult)

        # =========== Stage 2: routing ===========
        for it in range(ROUTING):
            p_c = bigf.tile([128, HB, CO, DO], F32, name="p_c", tag="pbig", bufs=1)
            if it == 0:
                for hb in range(HB):
                    t1 = ring.tile([128, 4, CO, DO], BF16, name="ct1")
                    nc.vector.tensor_tensor(
                        t1[:], uhat[:, hb, 0:4], uhat[:, hb, 4:8], op=ALU.add
                    )
                    t2 = ring.tile([128, 2, CO, DO], BF16, name="ct2")
                    nc.vector.tensor_tensor(t2[:], t1[:, 0:2], t1[:, 2:4], op=ALU.add)
                    us0 = ring.tile([128, CO, DO], F32, name="us0")
                    nc.vector.tensor_tensor(us0[:], t2[:, 0], t2[:, 1], op=ALU.add)
                    r0b = r0_t[:, hb : hb + 1].broadcast_to([128, CO, DO])
                    nc.vector.tensor_tensor(p_c[:, hb], us0[:], r0b, op=ALU.mult)
            else:
                m0 = sm.tile([128, HB, CI], F32, name="m0")
                nc.vector.tensor_reduce(m0[:], b_t[:], axis=AX.X, op=ALU.max)
                bmax = sm.tile([128, HB, CI], F32, name="bmax")
                pools(m0, True, bmax)
                cs = bigf.tile([128, HB, CI, CO], F32, name="cs", tag="half", bufs=2)
                bm_b = bmax[:].unsqueeze(3).broadcast_to([128, HB, CI, CO])
                nc.vector.tensor_tensor(cs[:], b_t[:], bm_b, op=ALU.subtract)
                c_t = bigf.tile([128, HB, CI, CO], BF16, name="c_t", tag="qtr", bufs=2)
                nc.scalar.activation(c_t[:], cs[:], AF.Exp)
                s_t = sm.tile([128, HB, CI], F32, name="s_t")
                nc.vector.tensor_reduce(s_t[:], c_t[:], axis=AX.X, op=ALU.add)
                sumc = sm.tile([128, HB, CI], F32, name="sumc")
                pools(s_t, False, sumc)
                rcp = sm.tile([128, HB, CI], F32, name="rcp")
                nc.vector.reciprocal(rcp[:], sumc[:])
                r_t = bigf.tile([128, HB, CI, CO], BF16, name="r_t", tag="qtr", bufs=2)
                rcp_b = rcp[:].unsqueeze(3).broadcast_to([128, HB, CI, CO])
                nc.vector.tensor_tensor(r_t[:], c_t[:], rcp_b, op=ALU.mult)
                for hb in range(HB):
                    x = ring.tile([128, CI, CO, DO], BF16, name="x")
                    r_b = r_t[:, hb].unsqueeze(3).broadcast_to([128, CI, CO, DO])
                    nc.vector.tensor_tensor(x[:], uhat[:, hb], r_b, op=ALU.mult)
                    t1 = ring.tile([128, 4, CO, DO], BF16, name="ct1")
                    nc.vector.tensor_tensor(t1[:], x[:, 0:4], x[:, 4:8], op=ALU.add)
                    t2 = ring.tile([128, 2, CO, DO], BF16, name="ct2")
                    nc.vector.tensor_tensor(t2[:], t1[:, 0:2], t1[:, 2:4], op=ALU.add)
                    nc.vector.tensor_tensor(p_c[:, hb], t2[:, 0], t2[:, 1], op=ALU.add)

            if it < ROUTING - 1:
                v_bf = bigf.tile([128, HB, CO, DO], BF16, name="v_bf", tag="half", bufs=2)
                squash(p_c, v_bf, None)
                for hb in range(HB):
                    y = ring.tile([128, CI, CO, DO], BF16, name="y")
                    v_b = v_bf[:, hb].unsqueeze(1).broadcast_to([128, CI, CO, DO])
                    nc.vector.tensor_tensor(y[:], uhat[:, hb], v_b, op=ALU.mult)
                    e1 = ring.tile([128, CI, CO, 8], BF16, name="dt1")
                    nc.vector.tensor_tensor(
                        e1[:], y[:, :, :, 0:8], y[:, :, :, 8:16], op=ALU.add
                    )
                    e2 = ring.tile([128, CI, CO, 4], BF16, name="dt2")
                    nc.vector.tensor_tensor(
                        e2[:], e1[:, :, :, 0:4], e1[:, :, :, 4:8], op=ALU.add
                    )
                    e3 = ring.tile([128, CI, CO, 2], BF16, name="dt3")
                    nc.vector.tensor_tensor(
                        e3[:], e2[:, :, :, 0:2], e2[:, :, :, 2:4], op=ALU.add
                    )
                    if it == 0:
                        nc.vector.tensor_tensor(
                            b_t[:, hb], e3[:, :, :, 0], e3[:, :, :, 1], op=ALU.add
                        )
                    else:
                        db = ring.tile([128, CI, CO], BF16, name="db")
                        nc.vector.tensor_tensor(
                            db[:], e3[:, :, :, 0], e3[:, :, :, 1], op=ALU.add
                        )
                        nc.vector.tensor_tensor(
                            b_t[:, hb], b_t[:, hb], db[:], op=ALU.add
                        )
            else:
                v_f = p_c
                squash(p_c, None, v_f)
                nc.sync.dma_start(v_d[:], v_f[:])
    return nc


# ============================ host side ============================

_CACHE = {}


def _host_consts(w):
    w1 = np.zeros((128, CI, 256), ml_dtypes.bfloat16)
    w2 = np.zeros((72, CI, 256), ml_dtypes.bfloat16)
    # w: [Ci, Co*Do=256, Di, 5, 5] f32; lhsT row t*8+di, cols (ci, m)
    for t, (kh, kw) in enumerate(K1_TAPS):
        for di in range(DI):
            w1[t * 8 + di] = w[:, :, di, kh, kw].astype(ml_dtypes.bfloat16)
    for t, (kh, kw) in enumerate(K2_TAPS):
        for di in range(DI):
            w2[t * 8 + di] = w[:, :, di, kh, kw].astype(ml_dtypes.bfloat16)

    hw_cnt = np.zeros((H, W), np.float32)
    for h in range(H):
        for wv in range(W):
            ch = min(h + 2, H - 1) - max(h - 2, 0) + 1
            cw = min(wv + 2, W - 1) - max(wv - 2, 0) + 1
            hw_cnt[h, wv] = ch * cw
    r0 = 1.0 / (CO * hw_cnt)
    r0f = r0.reshape(HB, 128)  # hw = hb*128 + p
    r0c = np.ascontiguousarray(r0f.T)
    return w1, w2, r0c


def _get_nc():
    if "nc" not in _CACHE:
        nc = bacc.Bacc("TRN2", target_bir_lowering=False, debug=False, num_devices=8)
        _emit(nc)
        nc.compile()
        _CACHE["nc"] = nc
    return _CACHE["nc"]


def kernel(u, w):
    u = np.asarray(u, np.float32)
    N = u.shape[0]
    assert N == 8
    nc = _get_nc()
    w1, w2, r0c = _host_consts(np.asarray(w, np.float32))
    in_maps = [
        {"u": u[n].astype(ml_dtypes.bfloat16), "w1": w1, "w2": w2, "r0c": r0c}
        for n in range(N)
    ]
    res = run_bass_kernel_spmd(nc, in_maps, core_ids=list(range(N)))
    out = np.stack([res.results[n]["v"] for n in range(N)])  # [8, 128, HB, CO, DO]
    out = out.transpose(0, 3, 4, 2, 1)  # n co do hb p  (hw = hb*128 + p)
    return np.ascontiguousarray(out.reshape(N, CO, DO, H, W), dtype=np.float32)



# revision 7
# speedup vs baseline: 2.0432x; 2.0432x over previous
"""Trainium2 Bass kernel for nn_CapsuleLayer (grouped 5x5 capsule conv + 3-iter
dynamic routing with local softmax), data-parallel over batch N=8 across 8 cores.

Layout: spatial positions on SBUF partitions, channels on free dims.
  hw = hb*128 + p  (raster order), hb in [0,18), p in [0,128)
  uhat: [p=128, (hb=18, ci=8, do=16, co=16)] bf16.  co innermost keeps packed
  bf16 tensor_tensor ops in the DVE 2x perf mode; broadcasts of r (over do)
  and v (over ci) are middle-dim stride-0, which preserves the fast mode.

Conv: host-side im2col (pure layout transform) stages tap-expanded lhsT pats
in DRAM; per ci one [128,HW] + one [72,HW] load, then per hb two matmuls
(K=128 taps*di, K=72) accumulate in PSUM; evacuation rotates Act/DVE/Pool.

Routing: all channel contractions are free-dim tensor-op trees in bf16, with
each big op range-split between DVE and Pool (gpsimd) so both engines run in
parallel.  The 5x5 spatial pools run in an h-on-partitions layout: one DMA
reorder down+up per side, the separable 5-tap window max/sum done with
partition-shifted (h) and free-shifted (w) tensor_tensor trees.
"""

import numpy as np
import ml_dtypes
from contextlib import ExitStack

import concourse.bass as bass
import concourse.tile as tile
from concourse import bacc, mybir
from concourse.bass_utils import run_bass_kernel_spmd

F32 = mybir.dt.float32
BF16 = mybir.dt.bfloat16
AF = mybir.ActivationFunctionType
ALU = mybir.AluOpType

CI, DI, CO, DO = 8, 8, 16, 16
H = W = 48
HW = H * W
HB = 18
ROUTING = 3
NEG = -3.0e38

# hb chunks for the big ops; within each chunk the last POOL_P hb go to the
# Pool engine (gpsimd), the rest to DVE.
CHUNKS = [(0, 6), (6, 12), (12, 18)]
POOL_P = 1


def _emit(nc):
    p1_d = nc.dram_tensor("p1", [CI, 128, HW], BF16, kind="ExternalInput").ap()
    p2_d = nc.dram_tensor("p2", [CI, 72, HW], BF16, kind="ExternalInput").ap()
    w1_d = nc.dram_tensor("w1", [128, CI, 256], BF16, kind="ExternalInput").ap()
    w2_d = nc.dram_tensor("w2", [72, CI, 256], BF16, kind="ExternalInput").ap()
    r0_d = nc.dram_tensor("r0c", [128, HB], F32, kind="ExternalInput").ap()
    v_d = nc.dram_tensor("v", [128, HB, DO, CO], BF16, kind="ExternalOutput").ap()

    with tile.TileContext(nc) as tc, ExitStack() as ctx:
        const = ctx.enter_context(tc.tile_pool(name="const", bufs=1))
        patp = ctx.enter_context(tc.tile_pool(name="patp", bufs=1))
        psum = ctx.enter_context(tc.tile_pool(name="psum", bufs=8, space="PSUM"))
        big = ctx.enter_context(tc.tile_pool(name="big", bufs=1))
        scr = ctx.enter_context(tc.tile_pool(name="scr", bufs=1))
        sm = ctx.enter_context(tc.tile_pool(name="sm", bufs=1))
        poolt = ctx.enter_context(tc.tile_pool(name="poolt", bufs=1))
        dpool = ctx.enter_context(tc.tile_pool(name="dpool", bufs=2, space="DRAM"))

        # ---- persistent tiles ----
        uhat = big.tile([128, HB, CI, DO, CO], BF16, name="uhat")
        b_t = big.tile([128, HB, CI, CO], F32, name="b_t")
        p_t = big.tile([128, HB, DO, CO], BF16, name="p_t")
        v_bf = big.tile([128, HB, DO, CO], BF16, name="v_bf")
        c_t = big.tile([128, HB, CI, CO], BF16, name="c_t")
        w1_t = const.tile([128, CI, 256], BF16, name="w1_t")
        w2_t = const.tile([72, CI, 256], BF16, name="w2_t")
        r0_t = const.tile([128, HB], F32, name="r0_t")
        eps_t = const.tile([128, 1], F32, name="eps_t")
        nc.sync.dma_start(w1_t[:], w1_d[:])
        nc.sync.dma_start(w2_t[:], w2_d[:])
        nc.sync.dma_start(r0_t[:], r0_d[:])
        nc.vector.memset(eps_t[:], 1e-9)

        # pool scratch.  W direction runs h-on-partitions with free-dim
        # shifts; H direction via 5 row-shifted DRAM re-reads (engines cannot
        # shift across partitions).  wp w-pad columns re-set per call.
        wp = poolt.tile([48, 52, CI], F32, name="wp")
        wt1 = poolt.tile([48, 51, CI], F32, name="wt1", tag="w1")
        wt2 = poolt.tile([48, 49, CI], F32, name="wt2", tag="w2")
        wt3 = poolt.tile([48, 48, CI], F32, name="wt3", tag="w3")
        hsh = poolt.tile([128, 5, HB, CI], F32, name="hsh")
        q1 = poolt.tile([128, HB, CI], F32, name="q1", tag="q1")
        q2 = poolt.tile([128, HB, CI], F32, name="q2", tag="q2")
        # DRAM row-padded buffers (2+48+2 rows) with guard rows written once
        gpad = poolt.tile([96, CI], F32, name="gpad")
        mdBM = dpool.tile([52 * W, CI], F32, name="mdBM", tag="mdBM", bufs=1)
        mdBS = dpool.tile([52 * W, CI], F32, name="mdBS", tag="mdBS", bufs=1)
        nc.vector.memset(gpad[:], NEG)
        nc.sync.dma_start(mdBM[0 : 2 * W], gpad[:])
        nc.sync.dma_start(mdBM[50 * W : 52 * W], gpad[:])
        nc.vector.memset(gpad[:], 0.0)
        nc.sync.dma_start(mdBS[0 : 2 * W], gpad[:])
        nc.sync.dma_start(mdBS[50 * W : 52 * W], gpad[:])

        # small persistent maps
        m0_t = sm.tile([128, HB, CI], F32, name="m0_t")
        bmax_t = sm.tile([128, HB, CI], F32, name="bmax_t")
        s_t = sm.tile([128, HB, CI], F32, name="s_t")
        sumc_t = sm.tile([128, HB, CI], F32, name="sumc_t")
        rcp_t = sm.tile([128, HB, CI], F32, name="rcp_t")
        rcpb_t = sm.tile([128, HB, CI], BF16, name="rcpb_t")
        nsq_t = sm.tile([128, HB, CO], F32, name="nsq_t")
        rs_t = sm.tile([128, HB, CO], F32, name="rs_t")
        rd_t = sm.tile([128, HB, CO], F32, name="rd_t")
        g2b_t = sm.tile([128, HB, CO], BF16, name="g2b_t")

        # =========== Stage 1: conv -> uhat ===========
        EVAC = [nc.scalar, nc.vector]
        for ci in range(CI):
            pat1 = patp.tile([128, HW], BF16, name="pat1", tag="pat1", bufs=2)
            pat2 = patp.tile([72, HW], BF16, name="pat2", tag="pat2", bufs=2)
            nc.sync.dma_start(pat1[:], p1_d[ci])
            nc.sync.dma_start(pat2[:], p2_d[ci])
            for hb in range(HB):
                ps = psum.tile([128, 256], F32, name="ps")
                lhs1 = pat1[:, hb * 128 : (hb + 1) * 128]
                lhs2 = pat2[:, hb * 128 : (hb + 1) * 128]
                nc.tensor.matmul(ps[:], lhs1, w1_t[:, ci, :], start=True, stop=False)
                nc.tensor.matmul(ps[:], lhs2, w2_t[:, ci, :], start=False, stop=True)
                eng = EVAC[hb % 2]
                dst = uhat[:, hb, ci]
                src = ps[:].rearrange("p (d c) -> p d c", d=DO)
                if eng is nc.scalar:
                    nc.scalar.copy(dst, src)
                else:
                    eng.tensor_copy(dst, src)

        # =========== helpers ===========
        def tt_split(h0, h1, dst_f, a_f, b_f, op, pool_hb=POOL_P):
            """dst = a op b over hb range [h0,h1): DVE takes [h0,h1-pool_hb),
            Pool the rest.  *_f(lo,hi) -> AP view for that hb range."""
            d = h1 - pool_hb
            if d > h0:
                nc.vector.tensor_tensor(dst_f(h0, d), a_f(h0, d), b_f(h0, d), op=op)
            if pool_hb:
                nc.gpsimd.tensor_tensor(dst_f(d, h1), a_f(d, h1), b_f(d, h1), op=op)

        def pools(src, is_max, out):
            """src [128,(hb,ci)] f32 -> 5x5 'same' window max/sum -> out."""
            op = ALU.max if is_max else ALU.add
            pad = NEG if is_max else 0.0
            mdB = mdBM if is_max else mdBS
            nc.vector.memset(wp[:, 0:2], pad)
            nc.vector.memset(wp[:, 50:52], pad)
            md = dpool.tile([HW, CI], F32, name="pmd", tag="pmd")
            nc.sync.dma_start(
                md[:].rearrange("(hb p) ci -> p hb ci", hb=HB), src[:]
            )
            nc.sync.dma_start(
                wp[:, 2:50, :], md[:].rearrange("(h w) ci -> h w ci", h=H)
            )
            # w-direction 5-tap tree (free-dim shifts)
            nc.vector.tensor_tensor(wt1[:], wp[:, 0:51], wp[:, 1:52], op=op)
            nc.vector.tensor_tensor(wt2[:], wt1[:, 0:49], wt1[:, 2:51], op=op)
            nc.vector.tensor_tensor(wt3[:], wt2[:, 0:48], wp[:, 4:52], op=op)
            # h-direction: write rows into the padded DRAM buffer, read back 5
            # row-shifted copies, reduce.
            nc.sync.dma_start(
                mdB[2 * W : 50 * W].rearrange("(h w) ci -> h w ci", h=H), wt3[:]
            )
            for k in range(5):
                o = k * W
                nc.sync.dma_start(
                    hsh[:, k],
                    mdB[o : o + HW].rearrange("(hb p) ci -> p hb ci", hb=HB),
                )
            nc.vector.tensor_tensor(q1[:], hsh[:, 0], hsh[:, 1], op=op)
            nc.vector.tensor_tensor(q2[:], hsh[:, 2], hsh[:, 3], op=op)
            nc.vector.tensor_tensor(q1[:], q1[:], q2[:], op=op)
            nc.vector.tensor_tensor(out[:], q1[:], hsh[:, 4], op=op)

        # =========== Stage 2: routing ===========
        for it in range(ROUTING):
            last = it == ROUTING - 1
            if it == 0:
                # S = sum_ci uhat (bf16 tree); p = S * r0 (per-partition scalar)
                for (h0, h1) in CHUNKS:
                    n = h1 - h0
                    t1 = scr.tile([128, n, 4, DO, CO], BF16, name="t1", tag="B")
                    tt_split(h0, h1,
                             lambda a, b: t1[:, a - h0 : b - h0],
                             lambda a, b: uhat[:, a:b, 0:4],
                             lambda a, b: uhat[:, a:b, 4:8], ALU.add)
                    t2 = scr.tile([128, n, 2, DO, CO], BF16, name="t2", tag="C")
                    tt_split(h0, h1,
                             lambda a, b: t2[:, a - h0 : b - h0],
                             lambda a, b: t1[:, a - h0 : b - h0, 0:2],
                             lambda a, b: t1[:, a - h0 : b - h0, 2:4], ALU.add)
                    S = scr.tile([128, n, DO, CO], BF16, name="S", tag="D")
                    tt_split(h0, h1,
                             lambda a, b: S[:, a - h0 : b - h0],
                             lambda a, b: t2[:, a - h0 : b - h0, 0],
                             lambda a, b: t2[:, a - h0 : b - h0, 1], ALU.add)
                    for hb in range(h0, h1):
                        nc.vector.tensor_scalar(
                            p_t[:, hb], S[:, hb - h0], r0_t[:, hb : hb + 1],
                            None, op0=ALU.mult,
                        )
            else:
                pools(m0_t, True, bmax_t)
                # cs = b - bmax (f32), c = exp(cs) bf16 on Act
                cs = scr.tile([128, HB, CI, CO], F32, name="cs", tag="D")
                bm_b = bmax_t[:].unsqueeze(3).broadcast_to([128, HB, CI, CO])
                tt_split(0, HB,
                         lambda a, b: cs[:, a:b],
                         lambda a, b: b_t[:, a:b],
                         lambda a, b: bm_b[:, a:b], ALU.subtract, pool_hb=4)
                nc.scalar.activation(c_t[:], cs[:], AF.Exp)
                # s = sum_co c (bf16 tree over innermost co)
                s1 = scr.tile([128, HB, CI, 8], BF16, name="s1", tag="F")
                nc.vector.tensor_tensor(
                    s1[:], c_t[:, :, :, 0:8], c_t[:, :, :, 8:16], op=ALU.add
                )
                s2 = scr.tile([128, HB, CI, 4], BF16, name="s2", tag="G")
                nc.vector.tensor_tensor(
                    s2[:], s1[:, :, :, 0:4], s1[:, :, :, 4:8], op=ALU.add
                )
                s3 = scr.tile([128, HB, CI, 2], BF16, name="s3", tag="HH")
                nc.vector.tensor_tensor(
                    s3[:], s2[:, :, :, 0:2], s2[:, :, :, 2:4], op=ALU.add
                )
                nc.vector.tensor_tensor(
                    s_t[:], s3[:, :, :, 0], s3[:, :, :, 1], op=ALU.add
                )
                pools(s_t, False, sumc_t)
                nc.vector.reciprocal(rcp_t[:], sumc_t[:])
                nc.vector.tensor_copy(rcpb_t[:], rcp_t[:])
                # r = c * (1/sumc), in place on c_t
                rb = rcpb_t[:].unsqueeze(3).broadcast_to([128, HB, CI, CO])
                tt_split(0, HB,
                         lambda a, b: c_t[:, a:b],
                         lambda a, b: c_t[:, a:b],
                         lambda a, b: rb[:, a:b], ALU.mult, pool_hb=4)
                # x = r_b * uhat ; p = sum_ci x (bf16 trees, chunked)
                r_b = c_t[:].unsqueeze(3).broadcast_to([128, HB, CI, DO, CO])
                for (h0, h1) in CHUNKS:
                    n = h1 - h0
                    x = scr.tile([128, n, CI, DO, CO], BF16, name="x", tag="A")
                    tt_split(h0, h1,
                             lambda a, b: x[:, a - h0 : b - h0],
                             lambda a, b: uhat[:, a:b],
                             lambda a, b: r_b[:, a:b], ALU.mult)
                    t1 = scr.tile([128, n, 4, DO, CO], BF16, name="t1b", tag="B")
                    tt_split(h0, h1,
                             lambda a, b: t1[:, a - h0 : b - h0],
                             lambda a, b: x[:, a - h0 : b - h0, 0:4],
                             lambda a, b: x[:, a - h0 : b - h0, 4:8], ALU.add)
                    t2 = scr.tile([128, n, 2, DO, CO], BF16, name="t2b", tag="C")
                    tt_split(h0, h1,
                             lambda a, b: t2[:, a - h0 : b - h0],
                             lambda a, b: t1[:, a - h0 : b - h0, 0:2],
                             lambda a, b: t1[:, a - h0 : b - h0, 2:4], ALU.add)
                    tt_split(h0, h1,
                             lambda a, b: p_t[:, a:b],
                             lambda a, b: t2[:, a - h0 : b - h0, 0],
                             lambda a, b: t2[:, a - h0 : b - h0, 1], ALU.add)

            # ---- squash: p -> v ----
            sq = scr.tile([128, HB, DO, CO], BF16, name="sq", tag="D")
            nc.scalar.activation(sq[:], p_t[:], AF.Square)
            n1 = scr.tile([128, HB, 8, CO], BF16, name="n1", tag="E")
            nc.vector.tensor_tensor(n1[:], sq[:, :, 0:8], sq[:, :, 8:16], op=ALU.add)
            n2 = scr.tile([128, HB, 4, CO], BF16, name="n2", tag="F")
            nc.vector.tensor_tensor(n2[:], n1[:, :, 0:4], n1[:, :, 4:8], op=ALU.add)
            n3 = scr.tile([128, HB, 2, CO], BF16, name="n3", tag="G")
            nc.vector.tensor_tensor(n3[:], n2[:, :, 0:2], n2[:, :, 2:4], op=ALU.add)
            nc.vector.tensor_tensor(nsq_t[:], n3[:, :, 0], n3[:, :, 1], op=ALU.add)
            nc.scalar.activation(rs_t[:], nsq_t[:], AF.Sqrt, bias=eps_t[:])
            nc.vector.scalar_tensor_tensor(
                rs_t[:], nsq_t[:], 1.0, rs_t[:], op0=ALU.add, op1=ALU.mult
            )
            nc.vector.reciprocal(rd_t[:], rs_t[:])
            nc.vector.tensor_tensor(nsq_t[:], nsq_t[:], rd_t[:], op=ALU.mult)
            nc.vector.tensor_copy(g2b_t[:], nsq_t[:])
            g_b = g2b_t[:].unsqueeze(2).broadcast_to([128, HB, DO, CO])
            tt_split(0, HB,
                     lambda a, b: v_bf[:, a:b],
                     lambda a, b: p_t[:, a:b],
                     lambda a, b: g_b[:, a:b], ALU.mult, pool_hb=4)

            if not last:
                # delta_b = sum_do uhat*v; bf16 trees over do (dim 3), chunked;
                # the m0 = max_co b tree for the next iteration is interleaved
                # per chunk so its down-DMA can fire early.
                v_b = v_bf[:].unsqueeze(2).broadcast_to([128, HB, CI, DO, CO])
                for (h0, h1) in CHUNKS:
                    n = h1 - h0
                    y = scr.tile([128, n, CI, DO, CO], BF16, name="y", tag="A")
                    tt_split(h0, h1,
                             lambda a, b: y[:, a - h0 : b - h0],
                             lambda a, b: uhat[:, a:b],
                             lambda a, b: v_b[:, a:b], ALU.mult)
                    e1 = scr.tile([128, n, CI, 8, CO], BF16, name="e1", tag="B")
                    tt_split(h0, h1,
                             lambda a, b: e1[:, a - h0 : b - h0],
                             lambda a, b: y[:, a - h0 : b - h0, :, 0:8],
                             lambda a, b: y[:, a - h0 : b - h0, :, 8:16], ALU.add)
                    e2 = scr.tile([128, n, CI, 4, CO], BF16, name="e2", tag="C")
                    tt_split(h0, h1,
                             lambda a, b: e2[:, a - h0 : b - h0],
                             lambda a, b: e1[:, a - h0 : b - h0, :, 0:4],
                             lambda a, b: e1[:, a - h0 : b - h0, :, 4:8], ALU.add)
                    e3 = scr.tile([128, n, CI, 2, CO], BF16, name="e3", tag="D")
                    tt_split(h0, h1,
                             lambda a, b: e3[:, a - h0 : b - h0],
                             lambda a, b: e2[:, a - h0 : b - h0, :, 0:2],
                             lambda a, b: e2[:, a - h0 : b - h0, :, 2:4], ALU.add)
                    if it == 0:
                        nc.vector.tensor_tensor(
                            b_t[:, h0:h1], e3[:, :, :, 0], e3[:, :, :, 1],
                            op=ALU.add,
                        )
                    else:
                        db = scr.tile([128, n, CI, CO], BF16, name="db", tag="E")
                        nc.vector.tensor_tensor(
                            db[:], e3[:, :, :, 0], e3[:, :, :, 1], op=ALU.add
                        )
                        nc.vector.tensor_tensor(
                            b_t[:, h0:h1], b_t[:, h0:h1], db[:], op=ALU.add
                        )
                    # m0 chunk for next iteration's softmax
                    u1 = scr.tile([128, n, CI, 8], F32, name="u1", tag="E")
                    nc.vector.tensor_tensor(
                        u1[:], b_t[:, h0:h1, :, 0:8], b_t[:, h0:h1, :, 8:16],
                        op=ALU.max,
                    )
                    u2 = scr.tile([128, n, CI, 4], F32, name="u2", tag="F")
                    nc.vector.tensor_tensor(
                        u2[:], u1[:, :, :, 0:4], u1[:, :, :, 4:8], op=ALU.max
                    )
                    u3 = scr.tile([128, n, CI, 2], F32, name="u3", tag="G")
                    nc.vector.tensor_tensor(
                        u3[:], u2[:, :, :, 0:2], u2[:, :, :, 2:4], op=ALU.max
                    )
                    nc.vector.tensor_tensor(
                        m0_t[:, h0:h1], u3[:, :, :, 0], u3[:, :, :, 1],
                        op=ALU.max,
                    )
            else:
                nc.sync.dma_start(v_d[:], v_bf[:])
    return nc


# ============================ host side ============================

_CACHE = {}


def _host_consts(w):
    # w: [Ci, Co*Do, Di, 5, 5] f32, channel index = co*16+do.
    # Conv lhsT rows: pat1 row = di*16 + kh*4 + kw (kh,kw in 0..4);
    # pat2 row = di*5 + kw for (kh=4, kw 0..5), then 40 + di*4 + kh for
    # (kh 0..4, kw=4).  Columns m = do*16 + co.
    w4 = w.reshape(CI, CO, DO, DI, 5, 5).transpose(3, 4, 5, 0, 2, 1)
    # w4: [di, kh, kw, ci, do, co]
    w4 = np.ascontiguousarray(w4).reshape(DI, 5, 5, CI, 256)
    w1 = np.ascontiguousarray(
        w4[:, 0:4, 0:4].reshape(128, CI, 256)
    ).astype(ml_dtypes.bfloat16)
    w2a = w4[:, 4, 0:5].reshape(40, CI, 256)
    w2b = w4[:, 0:4, 4].reshape(32, CI, 256)
    w2 = np.ascontiguousarray(np.concatenate([w2a, w2b], 0)).astype(
        ml_dtypes.bfloat16
    )

    hw_cnt = np.zeros((H, W), np.float32)
    for h in range(H):
        for wv in range(W):
            ch = min(h + 2, H - 1) - max(h - 2, 0) + 1
            cw = min(wv + 2, W - 1) - max(wv - 2, 0) + 1
            hw_cnt[h, wv] = ch * cw
    r0 = 1.0 / (CO * hw_cnt)
    r0c = np.ascontiguousarray(r0.reshape(HB, 128).T)
    return w1, w2, r0c


def _im2col(un):
    """un: [Ci, Di, H, W] bf16 -> pat1 [Ci, 128, HW], pat2 [Ci, 72, HW] bf16.
    Row layouts match _host_consts."""
    up = np.zeros((CI, DI, H + 4, W + 4), ml_dtypes.bfloat16)
    up[:, :, 2 : 2 + H, 2 : 2 + W] = un
    sw = np.lib.stride_tricks.sliding_window_view(up, (H, W), axis=(2, 3))
    # sw: [Ci, Di, 5, 5, H, W]
    p1 = sw[:, :, 0:4, 0:4].reshape(CI, DI * 16, HW)
    p2a = sw[:, :, 4, 0:5].reshape(CI, DI * 5, HW)
    p2b = sw[:, :, 0:4, 4].reshape(CI, DI * 4, HW)
    p2 = np.concatenate([p2a, p2b], 1)
    return np.ascontiguousarray(p1), np.ascontiguousarray(p2)


def _get_nc():
    if "nc" not in _CACHE:
        nc = bacc.Bacc("TRN2", target_bir_lowering=False, debug=False, num_devices=8)
        _emit(nc)
        nc.compile()
        _CACHE["nc"] = nc
    return _CACHE["nc"]


def kernel(u, w):
    u = np.asarray(u, np.float32)
    N = u.shape[0]
    assert N == 8
    nc = _get_nc()
    w1, w2, r0c = _host_consts(np.asarray(w, np.float32))
    ub = u.astype(ml_dtypes.bfloat16)
    in_maps = []
    for n in range(N):
        p1, p2 = _im2col(ub[n])
        in_maps.append({"p1": p1, "p2": p2, "w1": w1, "w2": w2, "r0c": r0c})
    res = run_bass_kernel_spmd(nc, in_maps, core_ids=list(range(N)))
    out = np.stack(
        [res.results[n]["v"].astype(np.float32) for n in range(N)]
    )  # [8, 128, HB, DO, CO]
    # hw = hb*128 + p ; out[n, co, do, h, w]
    out = out.transpose(0, 2, 1, 3, 4).reshape(N, HW, DO, CO)
    out = out.reshape(N, H, W, DO, CO).transpose(0, 4, 3, 1, 2)
    return np.ascontiguousarray(out, dtype=np.float32)


# revision 8
# speedup vs baseline: 2.3635x; 1.1567x over previous
"""Trainium2 Bass kernel for nn_CapsuleLayer (grouped 5x5 capsule conv + 3-iter
dynamic routing with local softmax), data-parallel over batch N=8 across 8 cores.

Layout: spatial positions on SBUF partitions, channels on free dims.
  hw = hb*128 + p  (raster order), hb in [0,18), p in [0,128)
  uhat: [p=128, (hb=18, ci=8, do=16, co=16)] bf16.  co innermost keeps packed
  bf16 tensor_tensor ops in the DVE 2x perf mode; broadcasts of r (over do)
  and v (over ci) are middle-dim stride-0, which preserves the fast mode.

Conv: host-side im2col (pure layout transform) stages tap-expanded lhsT pats
in DRAM; per ci one [128,HW] + one [72,HW] load, then per hb two matmuls
(K=128 taps*di, K=72) accumulate in PSUM; evacuation rotates Act/DVE/Pool.

Routing: all channel contractions are free-dim tensor-op trees in bf16, with
each big op range-split between DVE and Pool (gpsimd) so both engines run in
parallel.  The 5x5 spatial pools run in an h-on-partitions layout: one DMA
reorder down+up per side, the separable 5-tap window max/sum done with
partition-shifted (h) and free-shifted (w) tensor_tensor trees.
"""

import numpy as np
import ml_dtypes
from contextlib import ExitStack

import concourse.bass as bass
import concourse.tile as tile
from concourse import bacc, mybir
from concourse.bass_utils import run_bass_kernel_spmd

F32 = mybir.dt.float32
BF16 = mybir.dt.bfloat16
AF = mybir.ActivationFunctionType
ALU = mybir.AluOpType

CI, DI, CO, DO = 8, 8, 16, 16
H = W = 48
HW = H * W
HB = 18
ROUTING = 3
NEG = -3.0e38

# hb chunks for the big ops; within each chunk the last POOL_P hb go to the
# Pool engine (gpsimd), the rest to DVE.
CHUNKS = [(0, 6), (6, 12), (12, 18)]
POOL_P = 1


def _emit(nc):
    p1_d = nc.dram_tensor("p1", [CI, 128, HW], BF16, kind="ExternalInput").ap()
    p2_d = nc.dram_tensor("p2", [CI, 72, HW], BF16, kind="ExternalInput").ap()
    w1_d = nc.dram_tensor("w1", [128, CI, 256], BF16, kind="ExternalInput").ap()
    w2_d = nc.dram_tensor("w2", [72, CI, 256], BF16, kind="ExternalInput").ap()
    r0_d = nc.dram_tensor("r0c", [128, HB], F32, kind="ExternalInput").ap()
    v_d = nc.dram_tensor("v", [128, HB, DO, CO], BF16, kind="ExternalOutput").ap()

    with tile.TileContext(nc) as tc, ExitStack() as ctx:
        const = ctx.enter_context(tc.tile_pool(name="const", bufs=1))
        patp = ctx.enter_context(tc.tile_pool(name="patp", bufs=1))
        psum = ctx.enter_context(tc.tile_pool(name="psum", bufs=8, space="PSUM"))
        big = ctx.enter_context(tc.tile_pool(name="big", bufs=1))
        scr = ctx.enter_context(tc.tile_pool(name="scr", bufs=1))
        sm = ctx.enter_context(tc.tile_pool(name="sm", bufs=1))
        poolt = ctx.enter_context(tc.tile_pool(name="poolt", bufs=1))
        dpool = ctx.enter_context(tc.tile_pool(name="dpool", bufs=2, space="DRAM"))

        # ---- persistent tiles ----
        uhat = big.tile([128, HB, CI, DO, CO], BF16, name="uhat")
        b_t = big.tile([128, HB, CI, CO], F32, name="b_t")
        p_t = big.tile([128, HB, DO, CO], BF16, name="p_t")
        v_bf = big.tile([128, HB, DO, CO], BF16, name="v_bf")
        c_t = big.tile([128, HB, CI, CO], BF16, name="c_t")
        w1_t = const.tile([128, CI, 256], BF16, name="w1_t")
        w2_t = const.tile([72, CI, 256], BF16, name="w2_t")
        r0_t = const.tile([128, HB], F32, name="r0_t")
        eps_t = const.tile([128, 1], F32, name="eps_t")
        nc.sync.dma_start(w1_t[:], w1_d[:])
        nc.sync.dma_start(w2_t[:], w2_d[:])
        nc.sync.dma_start(r0_t[:], r0_d[:])
        nc.vector.memset(eps_t[:], 1e-9)

        # pool scratch.  W direction runs h-on-partitions with free-dim
        # shifts; H direction via 5 row-shifted DRAM re-reads (engines cannot
        # shift across partitions).  wp w-pad columns re-set per call.
        wp = poolt.tile([48, 52, CI], F32, name="wp")
        wt1 = poolt.tile([48, 51, CI], F32, name="wt1", tag="w1")
        wt2 = poolt.tile([48, 49, CI], F32, name="wt2", tag="w2")
        wt3 = poolt.tile([48, 48, CI], F32, name="wt3", tag="w3")
        hsh = poolt.tile([128, 5, HB, CI], F32, name="hsh")
        q1 = poolt.tile([128, HB, CI], F32, name="q1", tag="q1")
        q2 = poolt.tile([128, HB, CI], F32, name="q2", tag="q2")
        # DRAM row-padded buffers (2+48+2 rows) with guard rows written once
        gpad = poolt.tile([96, CI], F32, name="gpad")
        mdBM = dpool.tile([52 * W, CI], F32, name="mdBM", tag="mdBM", bufs=1)
        mdBS = dpool.tile([52 * W, CI], F32, name="mdBS", tag="mdBS", bufs=1)
        nc.vector.memset(gpad[:], NEG)
        nc.sync.dma_start(mdBM[0 : 2 * W], gpad[:])
        nc.sync.dma_start(mdBM[50 * W : 52 * W], gpad[:])
        nc.vector.memset(gpad[:], 0.0)
        nc.sync.dma_start(mdBS[0 : 2 * W], gpad[:])
        nc.sync.dma_start(mdBS[50 * W : 52 * W], gpad[:])

        # small persistent maps
        m0_t = sm.tile([128, HB, CI], F32, name="m0_t")
        bmax_t = sm.tile([128, HB, CI], F32, name="bmax_t")
        s_t = sm.tile([128, HB, CI], F32, name="s_t")
        sumc_t = sm.tile([128, HB, CI], F32, name="sumc_t")
        rcp_t = sm.tile([128, HB, CI], F32, name="rcp_t")
        rcpb_t = sm.tile([128, HB, CI], BF16, name="rcpb_t")
        nsq_t = sm.tile([128, HB, CO], F32, name="nsq_t")
        rs_t = sm.tile([128, HB, CO], F32, name="rs_t")
        rd_t = sm.tile([128, HB, CO], F32, name="rd_t")
        g2b_t = sm.tile([128, HB, CO], BF16, name="g2b_t")

        # =========== Stage 1: conv -> uhat ===========
        EVAC = [nc.scalar, nc.vector]
        for ci in range(CI):
            pat1 = patp.tile([128, HW], BF16, name="pat1", tag="pat1", bufs=2)
            pat2 = patp.tile([72, HW], BF16, name="pat2", tag="pat2", bufs=2)
            nc.sync.dma_start(pat1[:], p1_d[ci])
            nc.sync.dma_start(pat2[:], p2_d[ci])
            for hb in range(HB):
                ps = psum.tile([128, 256], F32, name="ps", tag="ps", bufs=6)
                lhs1 = pat1[:, hb * 128 : (hb + 1) * 128]
                lhs2 = pat2[:, hb * 128 : (hb + 1) * 128]
                nc.tensor.matmul(ps[:], lhs1, w1_t[:, ci, :], start=True, stop=False)
                nc.tensor.matmul(ps[:], lhs2, w2_t[:, ci, :], start=False, stop=True)
                eng = EVAC[hb % 2]
                dst = uhat[:, hb, ci]
                src = ps[:].rearrange("p (d c) -> p d c", d=DO)
                if eng is nc.scalar:
                    nc.scalar.copy(dst, src)
                else:
                    eng.tensor_copy(dst, src)

        # =========== helpers ===========
        def tt_split(h0, h1, dst_f, a_f, b_f, op, pool_hb=POOL_P):
            """dst = a op b over hb range [h0,h1): DVE takes [h0,h1-pool_hb),
            Pool the rest.  *_f(lo,hi) -> AP view for that hb range."""
            d = h1 - pool_hb
            if d > h0:
                nc.vector.tensor_tensor(dst_f(h0, d), a_f(h0, d), b_f(h0, d), op=op)
            if pool_hb:
                nc.gpsimd.tensor_tensor(dst_f(d, h1), a_f(d, h1), b_f(d, h1), op=op)

        def pools(src, is_max, out):
            """src [128,(hb,ci)] f32 -> 5x5 'same' window max/sum -> out."""
            op = ALU.max if is_max else ALU.add
            pad = NEG if is_max else 0.0
            mdB = mdBM if is_max else mdBS
            nc.vector.memset(wp[:, 0:2], pad)
            nc.vector.memset(wp[:, 50:52], pad)
            md = dpool.tile([HW, CI], F32, name="pmd", tag="pmd")
            nc.sync.dma_start(
                md[:].rearrange("(hb p) ci -> p hb ci", hb=HB), src[:]
            )
            nc.sync.dma_start(
                wp[:, 2:50, :], md[:].rearrange("(h w) ci -> h w ci", h=H)
            )
            # w-direction 5-tap tree (free-dim shifts)
            nc.vector.tensor_tensor(wt1[:], wp[:, 0:51], wp[:, 1:52], op=op)
            nc.vector.tensor_tensor(wt2[:], wt1[:, 0:49], wt1[:, 2:51], op=op)
            nc.vector.tensor_tensor(wt3[:], wt2[:, 0:48], wp[:, 4:52], op=op)
            # h-direction: write rows into the padded DRAM buffer, read back 5
            # row-shifted copies, reduce.
            nc.sync.dma_start(
                mdB[2 * W : 50 * W].rearrange("(h w) ci -> h w ci", h=H), wt3[:]
            )
            for k in range(5):
                o = k * W
                nc.sync.dma_start(
                    hsh[:, k],
                    mdB[o : o + HW].rearrange("(hb p) ci -> p hb ci", hb=HB),
                )
            nc.vector.tensor_tensor(q1[:], hsh[:, 0], hsh[:, 1], op=op)
            nc.vector.tensor_tensor(q2[:], hsh[:, 2], hsh[:, 3], op=op)
            nc.vector.tensor_tensor(q1[:], q1[:], q2[:], op=op)
            nc.vector.tensor_tensor(out[:], q1[:], hsh[:, 4], op=op)

        # =========== Stage 2: routing ===========
        for it in range(ROUTING):
            last = it == ROUTING - 1
            if it == 0:
                # S = sum_ci uhat (bf16 tree); p = S * r0 (per-partition scalar)
                for (h0, h1) in CHUNKS:
                    n = h1 - h0
                    t1 = scr.tile([128, n, 4, DO, CO], BF16, name="t1", tag="B")
                    tt_split(h0, h1,
                             lambda a, b: t1[:, a - h0 : b - h0],
                             lambda a, b: uhat[:, a:b, 0:4],
                             lambda a, b: uhat[:, a:b, 4:8], ALU.add)
                    t2 = scr.tile([128, n, 2, DO, CO], BF16, name="t2", tag="C")
                    tt_split(h0, h1,
                             lambda a, b: t2[:, a - h0 : b - h0],
                             lambda a, b: t1[:, a - h0 : b - h0, 0:2],
                             lambda a, b: t1[:, a - h0 : b - h0, 2:4], ALU.add)
                    S = scr.tile([128, n, DO, CO], BF16, name="S", tag="D")
                    tt_split(h0, h1,
                             lambda a, b: S[:, a - h0 : b - h0],
                             lambda a, b: t2[:, a - h0 : b - h0, 0],
                             lambda a, b: t2[:, a - h0 : b - h0, 1], ALU.add)
                    for hb in range(h0, h1):
                        nc.vector.tensor_scalar(
                            p_t[:, hb], S[:, hb - h0], r0_t[:, hb : hb + 1],
                            None, op0=ALU.mult,
                        )
            else:
                pools(m0_t, True, bmax_t)
                # cs = b - bmax (f32), c = exp(cs) bf16 on Act
                cs = scr.tile([128, HB, CI, CO], F32, name="cs", tag="D")
                bm_b = bmax_t[:].unsqueeze(3).broadcast_to([128, HB, CI, CO])
                tt_split(0, HB,
                         lambda a, b: cs[:, a:b],
                         lambda a, b: b_t[:, a:b],
                         lambda a, b: bm_b[:, a:b], ALU.subtract, pool_hb=4)
                nc.scalar.activation(c_t[:], cs[:], AF.Exp)
                # s = sum_co c (bf16 tree over innermost co)
                s1 = scr.tile([128, HB, CI, 8], BF16, name="s1", tag="F")
                nc.vector.tensor_tensor(
                    s1[:], c_t[:, :, :, 0:8], c_t[:, :, :, 8:16], op=ALU.add
                )
                s2 = scr.tile([128, HB, CI, 4], BF16, name="s2", tag="G")
                nc.vector.tensor_tensor(
                    s2[:], s1[:, :, :, 0:4], s1[:, :, :, 4:8], op=ALU.add
                )
                s3 = scr.tile([128, HB, CI, 2], BF16, name="s3", tag="HH")
                nc.vector.tensor_tensor(
                    s3[:], s2[:, :, :, 0:2], s2[:, :, :, 2:4], op=ALU.add
                )
                nc.vector.tensor_tensor(
                    s_t[:], s3[:, :, :, 0], s3[:, :, :, 1], op=ALU.add
                )
                pools(s_t, False, sumc_t)
                nc.vector.reciprocal(rcp_t[:], sumc_t[:])
                nc.vector.tensor_copy(rcpb_t[:], rcp_t[:])
                # r = c * (1/sumc), in place on c_t
                rb = rcpb_t[:].unsqueeze(3).broadcast_to([128, HB, CI, CO])
                tt_split(0, HB,
                         lambda a, b: c_t[:, a:b],
                         lambda a, b: c_t[:, a:b],
                         lambda a, b: rb[:, a:b], ALU.mult, pool_hb=4)
                # x = r_b * uhat ; p = sum_ci x (bf16 trees, chunked)
                r_b = c_t[:].unsqueeze(3).broadcast_to([128, HB, CI, DO, CO])
                for (h0, h1) in CHUNKS:
                    n = h1 - h0
                    x = scr.tile([128, n, CI, DO, CO], BF16, name="x", tag="A")
                    tt_split(h0, h1,
                             lambda a, b: x[:, a - h0 : b - h0],
                             lambda a, b: uhat[:, a:b],
                             lambda a, b: r_b[:, a:b], ALU.mult)
                    t1 = scr.tile([128, n, 4, DO, CO], BF16, name="t1b", tag="B")
                    tt_split(h0, h1,
                             lambda a, b: t1[:, a - h0 : b - h0],
                             lambda a, b: x[:, a - h0 : b - h0, 0:4],
                             lambda a, b: x[:, a - h0 : b - h0, 4:8], ALU.add)
                    t2 = scr.tile([128, n, 2, DO, CO], BF16, name="t2b", tag="C")
                    tt_split(h0, h1,
                             lambda a, b: t2[:, a - h0 : b - h0],
                             lambda a, b: t1[:, a - h0 : b - h0, 0:2],
                             lambda a, b: t1[:, a - h0 : b - h0, 2:4], ALU.add)
                    tt_split(h0, h1,
                             lambda a, b: p_t[:, a:b],
                             lambda a, b: t2[:, a - h0 : b - h0, 0],
                             lambda a, b: t2[:, a - h0 : b - h0, 1], ALU.add)

            # ---- squash: p -> v ----
            sq = scr.tile([128, HB, DO, CO], BF16, name="sq", tag="D")
            nc.scalar.activation(sq[:], p_t[:], AF.Square)
            n1 = scr.tile([128, HB, 8, CO], BF16, name="n1", tag="E")
            nc.vector.tensor_tensor(n1[:], sq[:, :, 0:8], sq[:, :, 8:16], op=ALU.add)
            n2 = scr.tile([128, HB, 4, CO], BF16, name="n2", tag="F")
            nc.vector.tensor_tensor(n2[:], n1[:, :, 0:4], n1[:, :, 4:8], op=ALU.add)
            n3 = scr.tile([128, HB, 2, CO], BF16, name="n3", tag="G")
            nc.vector.tensor_tensor(n3[:], n2[:, :, 0:2], n2[:, :, 2:4], op=ALU.add)
            nc.vector.tensor_tensor(nsq_t[:], n3[:, :, 0], n3[:, :, 1], op=ALU.add)
            nc.scalar.activation(rs_t[:], nsq_t[:], AF.Sqrt, bias=eps_t[:])
            nc.vector.scalar_tensor_tensor(
                rs_t[:], nsq_t[:], 1.0, rs_t[:], op0=ALU.add, op1=ALU.mult
            )
            nc.vector.reciprocal(rd_t[:], rs_t[:])
            nc.vector.tensor_tensor(nsq_t[:], nsq_t[:], rd_t[:], op=ALU.mult)
            nc.vector.tensor_copy(g2b_t[:], nsq_t[:])
            g_b = g2b_t[:].unsqueeze(2).broadcast_to([128, HB, DO, CO])
            tt_split(0, HB,
                     lambda a, b: v_bf[:, a:b],
                     lambda a, b: p_t[:, a:b],
                     lambda a, b: g_b[:, a:b], ALU.mult, pool_hb=4)

            if not last:
                # delta_b = sum_do uhat*v; bf16 trees over do (dim 3), chunked;
                # the m0 = max_co b tree for the next iteration is interleaved
                # per chunk so its down-DMA can fire early.
                v_b = v_bf[:].unsqueeze(2).broadcast_to([128, HB, CI, DO, CO])
                for (h0, h1) in CHUNKS:
                    n = h1 - h0
                    y = scr.tile([128, n, CI, DO, CO], BF16, name="y", tag="A")
                    tt_split(h0, h1,
                             lambda a, b: y[:, a - h0 : b - h0],
                             lambda a, b: uhat[:, a:b],
                             lambda a, b: v_b[:, a:b], ALU.mult)
                    e1 = scr.tile([128, n, CI, 8, CO], BF16, name="e1", tag="B")
                    tt_split(h0, h1,
                             lambda a, b: e1[:, a - h0 : b - h0],
                             lambda a, b: y[:, a - h0 : b - h0, :, 0:8],
                             lambda a, b: y[:, a - h0 : b - h0, :, 8:16], ALU.add)
                    e2 = scr.tile([128, n, CI, 4, CO], BF16, name="e2", tag="C")
                    tt_split(h0, h1,
                             lambda a, b: e2[:, a - h0 : b - h0],
                             lambda a, b: e1[:, a - h0 : b - h0, :, 0:4],
                             lambda a, b: e1[:, a - h0 : b - h0, :, 4:8], ALU.add)
                    e3 = scr.tile([128, n, CI, 2, CO], BF16, name="e3", tag="D")
                    tt_split(h0, h1,
                             lambda a, b: e3[:, a - h0 : b - h0],
                             lambda a, b: e2[:, a - h0 : b - h0, :, 0:2],
                             lambda a, b: e2[:, a - h0 : b - h0, :, 2:4], ALU.add)
                    if it == 0:
                        nc.vector.tensor_tensor(
                            b_t[:, h0:h1], e3[:, :, :, 0], e3[:, :, :, 1],
                            op=ALU.add,
                        )
                    else:
                        db = scr.tile([128, n, CI, CO], BF16, name="db", tag="E")
                        nc.vector.tensor_tensor(
                            db[:], e3[:, :, :, 0], e3[:, :, :, 1], op=ALU.add
                        )
                        nc.vector.tensor_tensor(
                            b_t[:, h0:h1], b_t[:, h0:h1], db[:], op=ALU.add
                        )
                    # m0 chunk for next iteration's softmax
                    u1 = scr.tile([128, n, CI, 8], F32, name="u1", tag="E")
                    nc.vector.tensor_tensor(
                        u1[:], b_t[:, h0:h1, :, 0:8], b_t[:, h0:h1, :, 8:16],
                        op=ALU.max,
                    )
                    u2 = scr.tile([128, n, CI, 4], F32, name="u2", tag="F")
                    nc.vector.tensor_tensor(
                        u2[:], u1[:, :, :, 0:4], u1[:, :, :, 4:8], op=ALU.max
                    )
                    u3 = scr.tile([128, n, CI, 2], F32, name="u3", tag="G")
                    nc.vector.tensor_tensor(
                        u3[:], u2[:, :, :, 0:2], u2[:, :, :, 2:4], op=ALU.max
                    )
                    nc.vector.tensor_tensor(
                        m0_t[:, h0:h1], u3[:, :, :, 0], u3[:, :, :, 1],
                        op=ALU.max,
                    )
            else:
                nc.sync.dma_start(v_d[:], v_bf[:])
    return nc


# ============================ host side ============================

_CACHE = {}


def _host_consts(w):
    # w: [Ci, Co*Do, Di, 5, 5] f32, channel index = co*16+do.
    # Conv lhsT rows: pat1 row = di*16 + kh*4 + kw (kh,kw in 0..4);
    # pat2 row = di*5 + kw for (kh=4, kw 0..5), then 40 + di*4 + kh for
    # (kh 0..4, kw=4).  Columns m = do*16 + co.
    w4 = w.reshape(CI, CO, DO, DI, 5, 5).transpose(3, 4, 5, 0, 2, 1)
    # w4: [di, kh, kw, ci, do, co]
    w4 = np.ascontiguousarray(w4).reshape(DI, 5, 5, CI, 256)
    w1 = np.ascontiguousarray(
        w4[:, 0:4, 0:4].reshape(128, CI, 256)
    ).astype(ml_dtypes.bfloat16)
    w2a = w4[:, 4, 0:5].reshape(40, CI, 256)
    w2b = w4[:, 0:4, 4].reshape(32, CI, 256)
    w2 = np.ascontiguousarray(np.concatenate([w2a, w2b], 0)).astype(
        ml_dtypes.bfloat16
    )

    hw_cnt = np.zeros((H, W), np.float32)
    for h in range(H):
        for wv in range(W):
            ch = min(h + 2, H - 1) - max(h - 2, 0) + 1
            cw = min(wv + 2, W - 1) - max(wv - 2, 0) + 1
            hw_cnt[h, wv] = ch * cw
    r0 = 1.0 / (CO * hw_cnt)
    r0c = np.ascontiguousarray(r0.reshape(HB, 128).T)
    return w1, w2, r0c


def _im2col(un):
    """un: [Ci, Di, H, W] bf16 -> pat1 [Ci, 128, HW], pat2 [Ci, 72, HW] bf16.
    Row layouts match _host_consts."""
    up = np.zeros((CI, DI, H + 4, W + 4), ml_dtypes.bfloat16)
    up[:, :, 2 : 2 + H, 2 : 2 + W] = un
    sw = np.lib.stride_tricks.sliding_window_view(up, (H, W), axis=(2, 3))
    # sw: [Ci, Di, 5, 5, H, W]
    p1 = sw[:, :, 0:4, 0:4].reshape(CI, DI * 16, HW)
    p2a = sw[:, :, 4, 0:5].reshape(CI, DI * 5, HW)
    p2b = sw[:, :, 0:4, 4].reshape(CI, DI * 4, HW)
    p2 = np.concatenate([p2a, p2b], 1)
    return np.ascontiguousarray(p1), np.ascontiguousarray(p2)


def _get_nc():
    if "nc" not in _CACHE:
        nc = bacc.Bacc("TRN2", target_bir_lowering=False, debug=False, num_devices=8)
        _emit(nc)
        nc.compile()
        _CACHE["nc"] = nc
    return _CACHE["nc"]


def kernel(u, w):
    u = np.asarray(u, np.float32)
    N = u.shape[0]
    assert N == 8
    nc = _get_nc()
    w1, w2, r0c = _host_consts(np.asarray(w, np.float32))
    ub = u.astype(ml_dtypes.bfloat16)
    in_maps = []
    for n in range(N):
        p1, p2 = _im2col(ub[n])
        in_maps.append({"p1": p1, "p2": p2, "w1": w1, "w2": w2, "r0c": r0c})
    res = run_bass_kernel_spmd(nc, in_maps, core_ids=list(range(N)))
    out = np.stack(
        [res.results[n]["v"].astype(np.float32) for n in range(N)]
    )  # [8, 128, HB, DO, CO]
    # hw = hb*128 + p ; out[n, co, do, h, w]
    out = out.transpose(0, 2, 1, 3, 4).reshape(N, HW, DO, CO)
    out = out.reshape(N, H, W, DO, CO).transpose(0, 4, 3, 1, 2)
    return np.ascontiguousarray(out, dtype=np.float32)
